# revision 23
# baseline (speedup 1.0000x reference)
"""Trainium2 Bass kernel for nn_NisuyNN_90434831384984.

Math: the reference's stack+reshape makes MLP row (s,t,b) depend only on s
(b in {0,1}) or only on t (b in {2,3}), and rows for b=2,3 equal those for
b=0,1 — so the 4096-row MLP collapses to 64 unique rows producing 64 unique
32x32 policy matrices.  The power iteration (50 steps, eigengap ~0.012)
is replaced by 13 unnormalized steps (converged below fp32 eps; the final
deltas use only intra-vector ratios, so the scale cancels).

Distribution: Megatron-style column-split of every layer across 8 cores
(each core owns a 512-wide slice of each hidden layer / 128-wide slice of
the output layer), with an AllGather of the (locally transposed)
activations between layers.  Weights are sliced on the host; each core
reads 1/8 of the weight bytes.
"""

import numpy as np

DIM = 128
N = 32
B = 4
H = 4096
NC = 8          # cores
SL = H // NC    # 512 hidden slice
OF = N * N      # 1024 output features
OSL = OF // NC  # 128 output slice
R = 64          # unique MLP rows
KC = 128        # contraction chunk
PI_ITERS = 12   # extra matvec iterations after the init row-sum step
SLOPE = 0.01

_COMPILED = None
LAST_RESULTS = None


def _build_body(nc, tc, tile, mybir, aps, debug=False, stage=99):
    f32 = mybir.dt.float32
    AF = mybir.ActivationFunctionType
    ALU = mybir.AluOpType
    AX = mybir.AxisListType
    rg = [list(range(NC))]

    from contextlib import ExitStack
    es = ExitStack()
    cpool = es.enter_context(tc.tile_pool(name="consts", bufs=1))
    wpool = es.enter_context(tc.tile_pool(name="w", bufs=2))
    bpool = es.enter_context(tc.tile_pool(name="b", bufs=2))
    apool = es.enter_context(tc.tile_pool(name="act", bufs=2))
    atp = es.enter_context(tc.tile_pool(name="atT", bufs=2))
    lpool = es.enter_context(tc.tile_pool(name="lhs", bufs=2))
    pipool = es.enter_context(tc.tile_pool(name="pi", bufs=2))
    tailp = es.enter_context(tc.tile_pool(name="tail", bufs=1))
    ps = es.enter_context(tc.tile_pool(name="ps", bufs=2, space="PSUM"))
    pst = es.enter_context(tc.tile_pool(name="pst", bufs=2, space="PSUM"))
    dram = es.enter_context(tc.tile_pool(name="dram", bufs=2, space="DRAM"))

    # ---- constants ----
    id64 = cpool.tile([64, 64], f32)
    nc.gpsimd.dma_start(id64[:], aps["ID64"][:])
    dmask = cpool.tile([R, N], f32)
    nc.gpsimd.dma_start(dmask[:], aps["DMASK"][:])
    t01 = cpool.tile([R, N], f32)
    nc.gpsimd.dma_start(t01[:], aps["T01"][:])
    tt23 = cpool.tile([R, N], f32)
    nc.gpsimd.dma_start(tt23[:], aps["TT23"][:])
    mac = cpool.tile([R, 2], f32)
    nc.gpsimd.dma_start(mac[:], aps["MAC"][:])
    ones = cpool.tile([1, R], f32)
    nc.vector.memset(ones[:], 1.0)

    def load_bias(name, width):
        bt = bpool.tile([1, width], f32, tag="bias")
        nc.gpsimd.dma_start(bt[0:1, :], aps[name].unsqueeze(0))
        return bt

    def mlp_layer(lhsT, nk, w_ap, b_name, width, func):
        """lhsT: SBUF tile [128, nk*64] of transposed activations.
        w_ap: DRAM AP [nk*128, width].  Returns SBUF [R, width] activations."""
        wt = wpool.tile([KC, nk * width], f32, tag="w")
        nc.sync.dma_start(
            wt[:].rearrange("p (c n) -> p c n", n=width),
            w_ap.rearrange("(c p) n -> p c n", p=KC),
        )
        bt = load_bias(b_name, width)
        pt = ps.tile([R, width], f32, tag="ps")
        for k in range(nk):
            nc.tensor.matmul(
                pt[:],
                lhsT[:, k * R:(k + 1) * R],
                wt[:, k * width:(k + 1) * width],
                start=(k == 0),
                stop=False,
            )
        nc.tensor.matmul(pt[:], ones[0:1, :], bt[0:1, :], start=False, stop=True)
        act = apool.tile([R, width], f32, tag="act")
        sc = apool.tile([R, width], f32, tag="lrelu_sc")
        nc.vector.tensor_scalar_mul(sc[:], pt[:], SLOPE)
        if func == "lrelu":
            nc.vector.tensor_tensor(act[:], pt[:], sc[:], op=ALU.max)
        else:  # lrelu then sigmoid (layer 6)
            lr = apool.tile([R, width], f32, tag="lrelu_out")
            nc.vector.tensor_tensor(lr[:], pt[:], sc[:], op=ALU.max)
            nc.scalar.activation(act[:], lr[:], AF.Sigmoid)
        return act

    def gather_transposed(act, width):
        """Transpose local [R, width] slice, AllGather, return SBUF lhsT tile
        [128, (NC*width/128)*64] ordered by global K-chunk."""
        nj = width // KC  # transposes (4)
        att = atp.tile([KC, nj * R], f32, tag="atT")
        for j in range(nj):
            tp = pst.tile([KC, R], f32, tag="pst")
            nc.tensor.transpose(tp[:], act[:, j * KC:(j + 1) * KC], id64[:])
            nc.vector.tensor_copy(att[:, j * R:(j + 1) * R], tp[:])
        ag_in = dram.tile([KC, nj * R], f32, tag="agin")
        nc.scalar.dma_start(ag_in[:], att[:])
        ag_out = dram.tile([NC * KC, nj * R], f32, tag="agout", addr_space="Shared")
        nc.gpsimd.collective_compute(
            "AllGather",
            ALU.bypass,
            replica_groups=rg,
            ins=[ag_in[:].opt()],
            outs=[ag_out[:].opt()],
        )
        nk = NC * width // KC  # total K-chunks of next layer (32)
        lt = lpool.tile([KC, nk * R], f32, tag="lhs")
        nc.scalar.dma_start(
            lt[:].rearrange("p (g j r) -> p g j r", g=NC, r=R),
            ag_out[:].rearrange("(g p) (j r) -> p g j r", p=KC, r=R),
        )
        return lt

    # ---- layer 1 (K = 256 = 2 chunks) ----
    xt = lpool.tile([KC, 2 * R], f32, tag="lhs")
    nc.scalar.dma_start(
        xt[:].rearrange("p (c r) -> p c r", r=R),
        aps["XT"].rearrange("(c p) r -> p c r", p=KC),
    )
    a1 = mlp_layer(xt, 2, aps["W1"], "b1", SL, "lrelu")
    if debug:
        nc.sync.dma_start(aps["dbg1"][:], a1[:])
    if stage == 1:
        es.close(); return
    lt = gather_transposed(a1, SL)
    if debug:
        nc.sync.dma_start(aps["dbglt1"][:], lt[:])
    if stage == 2:
        es.close(); return
    # ---- layers 2..5 ----
    for li in range(2, 6):
        a = mlp_layer(lt, H // KC, aps[f"W{li}"], f"b{li}", SL, "lrelu")
        if debug:
            nc.sync.dma_start(aps[f"dbg{li}"][:], a[:])
        lt = gather_transposed(a, SL)
    # ---- layer 6 + sigmoid + mult/add ----
    s6 = mlp_layer(lt, H // KC, aps["W6"], "b6", OSL, "sigmoid")
    p6 = apool.tile([R, OSL], f32, tag="p6")
    nc.vector.tensor_scalar(
        p6[:], s6[:], mac[:, 0:1], mac[:, 1:2], op0=ALU.mult, op1=ALU.add
    )
    if debug:
        nc.sync.dma_start(aps["dbg6"][:], p6[:])
    ag6_in = dram.tile([R, OSL], f32, tag="agin6")
    nc.scalar.dma_start(ag6_in[:], p6[:])
    ag6_out = dram.tile([NC * R, OSL], f32, tag="agout6", addr_space="Shared")
    nc.gpsimd.collective_compute(
        "AllGather",
        ALU.bypass,
        replica_groups=rg,
        ins=[ag6_in[:].opt()],
        outs=[ag6_out[:].opt()],
    )
    M = pipool.tile([R, OF], f32, tag="M")  # policy matrices, row p = 32x32 M_p
    nc.scalar.dma_start(
        M[:].rearrange("p (g n) -> p g n", g=NC),
        ag6_out[:].rearrange("(g p) n -> p g n", p=R),
    )
    if stage == 3:
        es.close(); return

    # ---- power iteration: b <- M b, unnormalized ----
    M3 = M[:].rearrange("p (r q) -> p r q", q=N)
    bv = pipool.tile([R, N], f32, tag="bv")
    nc.vector.reduce_sum(bv[:], M3, axis=AX.X)  # first step from b0 = ones
    tmp = pipool.tile([R, OF], f32, tag="pit")
    for _ in range(PI_ITERS):
        bb = bv[:].unsqueeze(1).broadcast_to((R, N, N))
        t3 = tmp[:].rearrange("p (r q) -> p r q", q=N)
        nc.vector.tensor_tensor(t3, M3, bb, op=mybir.AluOpType.mult)
        bv = pipool.tile([R, N], f32, tag="bv")
        nc.vector.reduce_sum(bv[:], t3, axis=AX.X)

    if stage == 4:
        es.close(); return
    # ---- deltas tail ----
    scr = tailp.tile([R, N], f32, tag="scr")
    d = tailp.tile([R, 1], f32, tag="d")
    nc.vector.tensor_tensor(scr[:], bv[:], dmask[:], op=ALU.mult)
    nc.vector.reduce_sum(d[:], scr[:], axis=AX.X)
    if stage == 41:
        es.close(); return
    recipd = tailp.tile([R, 1], f32, tag="rd")
    nc.vector.reciprocal(recipd[:], d[:])
    recipE = tailp.tile([R, N], f32, tag="rE")
    nc.vector.reciprocal(recipE[:], bv[:])
    w01 = tailp.tile([R, 1], f32, tag="w01")
    nc.vector.reduce_sum(w01[:], t01[:], axis=AX.X)
    coef_s = tailp.tile([R, 1], f32, tag="cs")
    nc.vector.tensor_tensor(coef_s[:], w01[:], recipd[:], op=ALU.mult)
    scr2 = tailp.tile([R, N], f32, tag="scr2")
    c23 = tailp.tile([R, 1], f32, tag="c23")
    nc.vector.tensor_tensor(scr2[:], tt23[:], recipE[:], op=ALU.mult)
    nc.vector.reduce_sum(c23[:], scr2[:], axis=AX.X)
    if stage == 42:
        es.close(); return
    coef = tailp.tile([R, B], f32, tag="coef")
    nc.vector.memset(coef[:], 0.0)
    nc.vector.tensor_copy(coef[0:32, 0:1], coef_s[0:32, :])
    nc.vector.tensor_copy(coef[32:64, 1:2], coef_s[32:64, :])
    nc.vector.tensor_copy(coef[0:32, 2:3], c23[0:32, :])
    nc.vector.tensor_copy(coef[32:64, 3:4], c23[32:64, :])
    if stage == 43:
        es.close(); return
    pd = pst.tile([B, N], f32, tag="pst")
    nc.tensor.matmul(pd[:], coef[:], bv[:], start=True, stop=True)
    osb = tailp.tile([B, N], f32, tag="osb")
    nc.vector.tensor_copy(osb[:], pd[:])
    nc.sync.dma_start(aps["out"][:], osb[:])
    es.close()


def build(debug=False, stage=99):
    import concourse.bacc as bacc
    import concourse.mybir as mybir
    import concourse.tile as tile

    f32 = mybir.dt.float32
    nc = bacc.Bacc("TRN2", target_bir_lowering=False, debug=False, num_devices=NC)
    shapes = {
        "XT": [2 * DIM, R],
        "W1": [2 * DIM, SL], "b1": [SL],
        "W2": [H, SL], "b2": [SL],
        "W3": [H, SL], "b3": [SL],
        "W4": [H, SL], "b4": [SL],
        "W5": [H, SL], "b5": [SL],
        "W6": [H, OSL], "b6": [OSL],
        "T01": [R, N], "TT23": [R, N],
        "DMASK": [R, N], "MAC": [R, 2], "ID64": [64, 64],
    }
    aps = {
        k: nc.dram_tensor(k, v, f32, kind="ExternalInput").ap()
        for k, v in shapes.items()
    }
    aps["out"] = nc.dram_tensor("out", [B, N], f32, kind="ExternalOutput").ap()
    if debug:
        for li in range(1, 6):
            aps[f"dbg{li}"] = nc.dram_tensor(
                f"dbg{li}", [R, SL], f32, kind="ExternalOutput"
            ).ap()
        aps["dbglt1"] = nc.dram_tensor(
            "dbglt1", [KC, 32 * R], f32, kind="ExternalOutput"
        ).ap()
        aps["dbg6"] = nc.dram_tensor("dbg6", [R, OSL], f32, kind="ExternalOutput").ap()
    with tile.TileContext(nc) as tc:
        _build_body(nc, tc, tile, mybir, aps, debug=debug, stage=stage)
    nc.compile()
    return nc


def prep_in_maps(inputs):
    f = np.float32
    E = np.asarray(inputs["batch_node_embeddings"], f)   # (B,N,D)
    T = np.asarray(inputs["batch_Ts"], f)                # (B,N,N)
    mult = np.asarray(inputs["mult_const_batch"], f).reshape(-1)[0]
    add = np.asarray(inputs["add_const_batch"], f).reshape(-1)[0]
    S = np.transpose(E, (1, 0, 2))                       # (N,B,D)
    G0 = np.concatenate([S[:, 0], S[:, 1]], axis=-1)     # (N, 2D)
    G1 = np.concatenate([S[:, 2], S[:, 3]], axis=-1)
    rows = np.concatenate([G0, G1], axis=0)              # (64, 256)
    common = {
        "XT": np.ascontiguousarray(rows.T),
        "T01": np.ascontiguousarray(np.concatenate([T[0], T[1]], axis=0)),
        "TT23": np.ascontiguousarray(np.concatenate([T[2].T, T[3].T], axis=0)),
        "DMASK": np.ascontiguousarray(np.tile(np.eye(N, dtype=f), (2, 1))),
        "MAC": np.ascontiguousarray(
            np.stack([np.full(R, mult, f), np.full(R, add, f)], axis=1)
        ),
        "ID64": np.eye(64, dtype=f),
    }
    in_maps = []
    for c in range(NC):
        m = dict(common)
        for li in range(1, 6):
            W = np.asarray(inputs[f"W{li}"], f)
            b = np.asarray(inputs[f"b{li}"], f)
            m[f"W{li}"] = np.ascontiguousarray(W[:, c * SL:(c + 1) * SL])
            m[f"b{li}"] = np.ascontiguousarray(b[c * SL:(c + 1) * SL])
        W6 = np.asarray(inputs["W6"], f)
        b6 = np.asarray(inputs["b6"], f)
        m["W6"] = np.ascontiguousarray(W6[:, c * OSL:(c + 1) * OSL])
        m["b6"] = np.ascontiguousarray(b6[c * OSL:(c + 1) * OSL])
        in_maps.append(m)
    return in_maps


def kernel(**inputs):
    global _COMPILED, LAST_RESULTS
    from concourse import bass_utils

    if _COMPILED is None:
        _COMPILED = build()
    in_maps = prep_in_maps(inputs)
    res = bass_utils.run_bass_kernel_spmd(
        _COMPILED, in_maps, core_ids=list(range(NC))
    )
    LAST_RESULTS = res
    return np.asarray(res.results[0]["out"], np.float32)
